# revision 2
# baseline (speedup 1.0000x reference)
"""3-layer GAT (2 heads x 128) on 8 TRN2 NeuronCores — Bass/Tile kernel.

Sharding: nodes partitioned across cores by destination (graph parallel);
weights replicated; per-layer AllGather of transposed features.

Device algorithm per layer:
  phase A' (local): esed[j] = hT_local_chunk.T @ W_ext[:,256:260]  (es/ed)
  phase A (replicated): table[p] = h[p] @ W_ext  -> [xh(256), es, ed] rows
  phase B (sharded, per 128-dst block):
    dma_gather rows by src (2 src-half buckets, int16 idx), dma_gather
    ed by dst; ex = exp(max(t, .2t)), msg = ex*xh; PE matmul with one-hot
    S tiles accumulates [128 dst, 258] (agg heads + denominators);
    out = (agg0/den0 + agg1/den1)/2 + b  (+ELU); PE-transpose -> hT shard.
  AllGather hT shards between layers. Softmax max-subtraction is skipped
  (attention logits are O(1) here; exp is safe in fp32).
"""
import dataclasses
import numpy as np

import concourse.bass as bass
import concourse.bacc as bacc
import concourse.mybir as mybir
import concourse.tile as tile

f32 = mybir.dt.float32
f32r = mybir.dt.bfloat16  # bf16 matmul operands (fp32r broken on HW)
i16 = mybir.dt.int16
ALU = mybir.AluOpType
ACTF = mybir.ActivationFunctionType


@dataclasses.dataclass(frozen=True)
class Cfg:
    n: int = 50000
    ncores: int = 8
    t_bkt: int = 10
    nlayers: int = 3
    hid: int = 128          # per-head dim == in feat dim == 128 (fixed)

    @property
    def nb(self):  return self.n // self.ncores
    @property
    def cpb(self):  return (self.nb + 127) // 128
    @property
    def npc(self):  return self.cpb * 128
    @property
    def npad(self): return self.ncores * self.npc
    @property
    def half(self): return self.npad // 2
    @property
    def tpb(self):  return 2 * self.t_bkt
    @property
    def kb(self):   return self.t_bkt * 128


ROW = 384
EROW = 64


# ---------------------------------------------------------------- host side

def pack_nodes(cfg, deg):
    """perm [N] -> padded slot id. Cores by contiguous range; within a core,
    degree-sorted snake deal into cpb blocks (balances block edge counts)."""
    perm = np.full(cfg.n, -1, dtype=np.int64)
    for c in range(cfg.ncores):
        nodes = np.arange(c * cfg.nb, (c + 1) * cfg.nb)
        order = nodes[np.argsort(-deg[nodes], kind="stable")]
        blk = np.empty(cfg.nb, dtype=np.int64)
        slot = np.empty(cfg.nb, dtype=np.int64)
        fr = cfg.nb // cfg.cpb
        rem = cfg.nb - fr * cfg.cpb
        for r in range(fr):
            cols = np.arange(cfg.cpb)
            if r % 2:
                cols = cols[::-1]
            blk[r * cfg.cpb:(r + 1) * cfg.cpb] = cols
            slot[r * cfg.cpb:(r + 1) * cfg.cpb] = r
        if rem:
            cols = np.arange(rem) if fr % 2 == 0 else (cfg.cpb - 1 - np.arange(rem))
            blk[fr * cfg.cpb:] = cols
            slot[fr * cfg.cpb:] = fr
        perm[order] = c * cfg.npc + blk * 128 + slot
    return perm


def preprocess(cfg, edge_index):
    src0 = np.asarray(edge_index[0], dtype=np.int64)
    dst0 = np.asarray(edge_index[1], dtype=np.int64)
    loop = np.arange(cfg.n, dtype=np.int64)
    src = np.concatenate([src0, loop])
    dst = np.concatenate([dst0, loop])

    deg = np.bincount(dst, minlength=cfg.n)
    perm = pack_nodes(cfg, deg)

    psrc = perm[src]
    pdst = perm[dst]
    core = pdst // cfg.npc
    blk = (pdst % cfg.npc) // 128
    half = (psrc >= cfg.half).astype(np.int64)

    order = np.lexsort((psrc, half, blk, core))
    psrc, pdst, half = psrc[order], pdst[order], half[order]
    group = (core * cfg.cpb + blk)[order] * 2 + half

    ngroups = cfg.ncores * cfg.cpb * 2
    cnt = np.bincount(group, minlength=ngroups)
    t_need = int((cnt.max() + 127) // 128)
    assert cfg.t_bkt >= t_need, f"t_bkt={cfg.t_bkt} < needed {t_need}"
    starts = np.zeros(ngroups + 1, dtype=np.int64)
    np.cumsum(cnt, out=starts[1:])
    within = np.arange(len(group)) - starts[group]
    gpos = group * cfg.kb + within

    idx1 = np.zeros(ngroups * cfg.kb, dtype=np.int16)
    idx1[gpos] = (psrc - half * cfg.half).astype(np.int16)
    idx1 = idx1.reshape(cfg.ncores, cfg.cpb, 2 * cfg.kb)

    idx2 = np.zeros(ngroups * cfg.kb, dtype=np.int16)
    idx2[gpos] = (pdst % cfg.npc).astype(np.int16)
    idx2 = idx2.reshape(cfg.ncores, cfg.cpb, 2 * cfg.kb)

    sval = np.full(ngroups * cfg.kb, -1, dtype=np.int16)
    sval[gpos] = (pdst % 128).astype(np.int16)
    sval = sval.reshape(cfg.ncores, cfg.cpb, 2 * cfg.kb)
    return dict(perm=perm, t_need=t_need, idx1=idx1, idx2=idx2, sval=sval)


def wrap_rep(idx):
    """[..., K] int16 -> dma_gather wrapped layout [128, prod*K/16]."""
    K = idx.shape[-1]
    lead = int(np.prod(idx.shape[:-1]))
    w = idx.reshape(lead, K // 16, 16)
    w = np.transpose(w, (2, 0, 1)).reshape(16, lead * (K // 16))
    return np.tile(w, (8, 1)).copy()


def host_arrays(cfg, x, edge_index, params):
    import ml_dtypes
    bfl = ml_dtypes.bfloat16
    pp = preprocess(cfg, edge_index)
    perm = pp["perm"]

    xpad = np.zeros((cfg.npad, 128), dtype=np.float32)
    xpad[perm] = np.asarray(x, np.float32)
    xT_stack = np.ascontiguousarray(
        xpad.reshape(cfg.ncores, cfg.npc, 128).transpose(0, 2, 1)
        .reshape(cfg.ncores * 128, cfg.npc))

    w_ext = np.zeros((cfg.nlayers, 128, ROW), dtype=np.float32)
    bias = np.zeros((cfg.nlayers, 128, 128), dtype=np.float32)
    for li, (W, a_s, a_d, b) in enumerate(params):
        W = np.asarray(W, np.float32)
        w_ext[li, :, :256] = W
        w_ext[li, :, 256] = W[:, :128] @ np.asarray(a_s, np.float32)[0]
        w_ext[li, :, 257] = W[:, 128:] @ np.asarray(a_s, np.float32)[1]
        w_ext[li, :, 258] = W[:, :128] @ np.asarray(a_d, np.float32)[0]
        w_ext[li, :, 259] = W[:, 128:] @ np.asarray(a_d, np.float32)[1]
        bias[li] = np.tile(np.asarray(b, np.float32)[None, :], (128, 1))

    per_core = []
    for c in range(cfg.ncores):
        sv = pp["sval"][c].astype(np.int64)                  # [cpb, 2*kb]
        S = np.zeros((cfg.cpb, 128, cfg.tpb * 128), dtype=np.float32)
        bidx, eidx = np.nonzero(sv >= 0)
        t = eidx // 128
        e = eidx % 128
        S[bidx, e, t * 128 + sv[bidx, eidx]] = 1.0
        per_core.append(dict(
            xT_stack=xT_stack.astype(bfl),
            xT_local=np.ascontiguousarray(xT_stack[c * 128:(c + 1) * 128]).astype(bfl),
            w_ext=w_ext.astype(bfl), bias=bias,
            ident=np.eye(128, dtype=np.float32),
            idx1r=wrap_rep(pp["idx1"][c]),
            idx2r=wrap_rep(pp["idx2"][c]),
            s_tiles=S.astype(bfl),
        ))
    return pp, per_core


# -------------------------------------------------------------- device side

def build_nc(cfg):
    nc = bacc.Bacc("TRN2", num_devices=cfg.ncores)
    NPC, CPB, TPB, TB, KB, HALF = (cfg.npc, cfg.cpb, cfg.tpb, cfg.t_bkt,
                                   cfg.kb, cfg.half)
    NL = cfg.nlayers
    NSH = cfg.ncores          # shards
    HSH = NSH // 2            # shards per table half

    xT_stack = nc.dram_tensor("xT_stack", [NSH * 128, NPC], f32r, kind="ExternalInput")
    xT_local = nc.dram_tensor("xT_local", [128, NPC], f32r, kind="ExternalInput")
    w_ext_in = nc.dram_tensor("w_ext", [NL, 128, ROW], f32r, kind="ExternalInput")
    bias_in = nc.dram_tensor("bias", [NL, 128, 128], f32, kind="ExternalInput")
    ident_in = nc.dram_tensor("ident", [128, 128], f32, kind="ExternalInput")
    idx1_in = nc.dram_tensor("idx1r", [128, CPB * 2 * KB // 16], i16, kind="ExternalInput")
    idx2_in = nc.dram_tensor("idx2r", [128, CPB * 2 * KB // 16], i16, kind="ExternalInput")
    s_in = nc.dram_tensor("s_tiles", [CPB, 128, TPB * 128], f32r, kind="ExternalInput")
    out = nc.dram_tensor("out", [NPC, 128], f32, kind="ExternalOutput")

    with tile.TileContext(nc) as tc:
        with (
            tc.tile_pool(name="const", bufs=1) as constp,
            tc.tile_pool(name="dram", bufs=2, space="DRAM") as dramp,
        ):
            idx1_sb = constp.tile([128, CPB * 2 * KB // 16], i16)
            nc.sync.dma_start(idx1_sb[:], idx1_in.ap())
            idx2_sb = constp.tile([128, CPB * 2 * KB // 16], i16)
            nc.sync.dma_start(idx2_sb[:], idx2_in.ap())
            w_sb = constp.tile([128, NL * ROW], f32r)
            bias_sb = constp.tile([128, NL * 128], f32)
            for li in range(NL):
                nc.sync.dma_start(w_sb[:, li * ROW:(li + 1) * ROW], w_ext_in.ap()[li])
                nc.sync.dma_start(bias_sb[:, li * 128:(li + 1) * 128], bias_in.ap()[li])
            ident_sb = constp.tile([128, 128], f32)
            nc.sync.dma_start(ident_sb[:], ident_in.ap())

            greg1 = nc.gpsimd.to_reg(KB)
            greg2 = nc.gpsimd.to_reg(2 * KB)

            hT_ag = None      # DRAM [NSH*128, NPC]; None for layer 0
            hT_loc_dram = None

            for li in range(NL):
                w_l = w_sb[:, li * ROW:(li + 1) * ROW]
                bias_l = bias_sb[:, li * 128:(li + 1) * 128]
                last = li == NL - 1

                table = [dramp.tile([HSH * NPC, ROW], f32r, tag=f"tab{h}",
                                    name=f"table_l{li}_h{h}")
                         for h in range(2)]
                esed = dramp.tile([NPC, EROW], f32, tag="esed")

                # ---- phase A': local es/ed table
                with (
                    tc.tile_pool(name="slabL", bufs=1) as slabLp,
                    tc.tile_pool(name="aeps", bufs=4) as aepsp,
                    tc.tile_pool(name="psumE", bufs=4, space="PSUM") as psumEp,
                ):
                    hTl = slabLp.tile([128, NPC], f32r)
                    if li == 0:
                        nc.sync.dma_start(hTl[:], xT_local.ap())
                    else:
                        nc.sync.dma_start(hTl[:], hT_loc_dram[:])
                    for j in range(CPB):
                        psE = psumEp.tile([128, 4], f32)
                        nc.tensor.matmul(
                            psE[:],
                            hTl[:, j * 128:(j + 1) * 128],
                            w_l[:, 256:260],
                            start=True, stop=True)
                        tE = aepsp.tile([128, 4], f32)
                        nc.vector.tensor_copy(tE[:], psE[:])
                        nc.sync.dma_start(esed[j * 128:(j + 1) * 128, 0:4], tE[:])

                # ---- phase A: full table (replicated)
                with (
                    tc.tile_pool(name="slabA", bufs=2) as slabAp,
                    tc.tile_pool(name="rowA", bufs=8) as rowAp,
                    tc.tile_pool(name="psumA", bufs=6, space="PSUM") as psumAp,
                ):
                    for s in range(NSH):
                        hTs = slabAp.tile([128, NPC], f32r)
                        if li == 0:
                            nc.sync.dma_start(
                                hTs[:], xT_stack.ap()[s * 128:(s + 1) * 128])
                        else:
                            nc.sync.dma_start(
                                hTs[:], hT_ag[s * 128:(s + 1) * 128])
                        tab = table[s // HSH]
                        base = (s % HSH) * NPC
                        for j in range(CPB):
                            psA = psumAp.tile([128, ROW], f32)
                            nc.tensor.matmul(
                                psA[:],
                                hTs[:, j * 128:(j + 1) * 128],
                                w_l,
                                start=True, stop=True)
                            tA = rowAp.tile([128, ROW], f32r)
                            nc.vector.tensor_copy(tA[:], psA[:])
                            nc.sync.dma_start(
                                tab[base + j * 128: base + (j + 1) * 128, :], tA[:])

                # ---- phase B
                with (
                    tc.tile_pool(name="g1", bufs=8) as g1p,
                    tc.tile_pool(name="g2", bufs=4) as g2p,
                    tc.tile_pool(name="sp", bufs=6) as sp,
                    tc.tile_pool(name="att", bufs=8) as attp,
                    tc.tile_pool(name="msgp", bufs=4) as msgp,
                    tc.tile_pool(name="ep", bufs=8) as epp,
                    tc.tile_pool(name="houtp", bufs=1) as houtp,
                    tc.tile_pool(name="psumB", bufs=3, space="PSUM") as psumBp,
                    tc.tile_pool(name="psumT", bufs=2, space="PSUM") as psumTp,
                ):
                    houtT = None
                    if not last:
                        houtT = houtp.tile([128, NPC], f32r)
                    for b in range(CPB):
                        psum = psumBp.tile([128, 258], f32)
                        g2 = g2p.tile([128, TPB, EROW], f32)
                        nc.gpsimd.dma_gather(
                            out_ap=g2[:], in_ap=esed,
                            idxs_ap=idx2_sb[:, b * 2 * KB // 16:
                                            (b + 1) * 2 * KB // 16],
                            num_idxs=2 * KB, num_idxs_reg=greg2,
                            elem_size=EROW, single_packet=False)
                        for h in range(2):
                            s_sb = sp.tile([128, TB * 128], f32r, name="s_sb")
                            nc.sync.dma_start(
                                s_sb[:],
                                s_in.ap()[b, :, h * TB * 128:(h + 1) * TB * 128])
                            g1 = g1p.tile([128, TB, ROW], f32r, name="g1")
                            nc.gpsimd.dma_gather(
                                out_ap=g1[:],
                                in_ap=table[h],
                                idxs_ap=idx1_sb[:, (b * 2 + h) * KB // 16:
                                                (b * 2 + h + 1) * KB // 16],
                                num_idxs=KB, num_idxs_reg=greg1,
                                elem_size=ROW, single_packet=False)
                            # attention scalars (batched across TB tiles)
                            a32 = attp.tile([128, TB, 2], f32, tag="a32")
                            nc.vector.tensor_copy(a32[:], g1[:, :, 256:258])
                            tat = attp.tile([128, TB, 2], f32, tag="tat")
                            nc.vector.tensor_tensor(
                                out=tat[:], in0=a32[:],
                                in1=g2[:, h * TB:(h + 1) * TB, 2:4], op=ALU.add)
                            lk = attp.tile([128, TB, 2], f32, tag="lk")
                            nc.vector.tensor_scalar(
                                out=lk[:], in0=tat[:], scalar1=0.2,
                                scalar2=None, op0=ALU.mult)
                            nc.vector.tensor_tensor(
                                out=lk[:], in0=lk[:], in1=tat[:], op=ALU.max)
                            exe = attp.tile([128, TB, 2], f32, tag="exe")
                            nc.scalar.activation(exe[:], lk[:], ACTF.Exp)
                            exb = attp.tile([128, TB, 2], f32r, tag="exb")
                            nc.vector.tensor_copy(exb[:], exe[:])
                            msg = msgp.tile([128, TB, 258], f32r, name="msg")
                            for hh in range(2):
                                nc.vector.tensor_tensor(
                                    out=msg[:, :, hh * 128:(hh + 1) * 128],
                                    in0=g1[:, :, hh * 128:(hh + 1) * 128],
                                    in1=exb[:, :, hh:hh + 1].broadcast_to(
                                        (128, TB, 128)),
                                    op=ALU.mult)
                            nc.vector.tensor_copy(msg[:, :, 256:258], exb[:])
                            for t in range(TB):
                                nc.tensor.matmul(
                                    psum[:],
                                    s_sb[:, t * 128:(t + 1) * 128],
                                    msg[:, t, :],
                                    start=(h == 0 and t == 0),
                                    stop=(h == 1 and t == TB - 1))
                        # epilogue
                        rec = attp.tile([128, 2], f32, tag="rec")
                        nc.vector.tensor_scalar(
                            out=rec[:], in0=psum[:, 256:258], scalar1=1e-30,
                            scalar2=None, op0=ALU.add)
                        nc.vector.reciprocal(rec[:], rec[:])
                        h_blk = epp.tile([128, 128], f32, tag="hblk")
                        nc.vector.tensor_scalar(
                            out=h_blk[:], in0=psum[:, 0:128],
                            scalar1=rec[:, 0:1], scalar2=0.5,
                            op0=ALU.mult, op1=ALU.mult)
                        m1 = epp.tile([128, 128], f32, tag="m1")
                        nc.vector.tensor_scalar(
                            out=m1[:], in0=psum[:, 128:256],
                            scalar1=rec[:, 1:2], scalar2=0.5,
                            op0=ALU.mult, op1=ALU.mult)
                        nc.vector.tensor_tensor(
                            out=h_blk[:], in0=h_blk[:], in1=m1[:], op=ALU.add)
                        nc.vector.tensor_tensor(
                            out=h_blk[:], in0=h_blk[:], in1=bias_l, op=ALU.add)
                        if not last:
                            # ELU = relu(x) + exp(min(x,0)) - 1
                            mn = epp.tile([128, 128], f32, tag="mn")
                            nc.vector.tensor_scalar(
                                out=mn[:], in0=h_blk[:], scalar1=0.0,
                                scalar2=None, op0=ALU.min)
                            emn = epp.tile([128, 128], f32, tag="emn")
                            nc.scalar.activation(emn[:], mn[:], ACTF.Exp)
                            nc.vector.tensor_scalar(
                                out=h_blk[:], in0=h_blk[:], scalar1=0.0,
                                scalar2=None, op0=ALU.max)
                            nc.vector.tensor_tensor(
                                out=h_blk[:], in0=h_blk[:], in1=emn[:],
                                op=ALU.add)
                            nc.vector.tensor_scalar(
                                out=h_blk[:], in0=h_blk[:], scalar1=-1.0,
                                scalar2=None, op0=ALU.add)
                            psT = psumTp.tile([128, 128], f32)
                            nc.tensor.transpose(psT[:], h_blk[:], ident_sb[:])
                            nc.vector.tensor_copy(
                                houtT[:, b * 128:(b + 1) * 128], psT[:])
                        else:
                            nc.sync.dma_start(
                                out[b * 128:(b + 1) * 128, :], h_blk[:])
                    if not last:
                        hT_loc_dram = dramp.tile([128, NPC], f32r, tag="hloc")
                        nc.sync.dma_start(hT_loc_dram[:], houtT[:])
                        hT_ag = dramp.tile([NSH * 128, NPC], f32r, tag="hag", addr_space="Shared")
                        nc.gpsimd.collective_compute(
                            "AllGather", ALU.bypass,
                            replica_groups=[list(range(cfg.ncores))],
                            ins=[hT_loc_dram.opt()], outs=[hT_ag.opt()])
    nc.compile()
    return nc


# ------------------------------------------------------------------ driver

def in_map(pc):
    return dict(xT_stack=pc["xT_stack"], xT_local=pc["xT_local"],
                w_ext=pc["w_ext"], bias=pc["bias"], ident=pc["ident"],
                idx1r=pc["idx1r"], idx2r=pc["idx2r"], s_tiles=pc["s_tiles"])


def run(cfg, x, edge_index, params, trace=False):
    from concourse.bass_utils import run_bass_kernel_spmd
    pp, per_core = host_arrays(cfg, x, edge_index, params)
    nc = build_nc(cfg)
    in_maps = [in_map(pc) for pc in per_core]
    res = run_bass_kernel_spmd(
        nc, in_maps, core_ids=list(range(cfg.ncores)), trace=trace)
    full = np.concatenate([res.results[c]["out"] for c in range(cfg.ncores)])
    return full[pp["perm"]], res


# ------------------------------------------------------------- entry point

_CFG = Cfg()


def kernel(x, edge_index, W0, a_src0, a_dst0, b0, W1, a_src1, a_dst1, b1,
           W2, a_src2, a_dst2, b2):
    """Full-input GAT kernel: shards across 8 NeuronCores internally."""
    params = [(W0, a_src0, a_dst0, b0), (W1, a_src1, a_dst1, b1),
              (W2, a_src2, a_dst2, b2)]
    cfg = _CFG
    try:
        out, _ = run(cfg, x, edge_index, params, trace=False)
    except AssertionError:
        pp = preprocess(dataclasses.replace(cfg, t_bkt=64), edge_index)
        cfg = dataclasses.replace(cfg, t_bkt=pp["t_need"])
        out, _ = run(cfg, x, edge_index, params, trace=False)
    return np.asarray(out, dtype=np.float32)



# revision 5
# speedup vs baseline: 1.2831x; 1.2831x over previous
"""3-layer GAT (2 heads x 128) on 8 TRN2 NeuronCores — Bass/Tile kernel, v2.

Sharding: nodes partitioned across cores by destination (graph parallel);
weights replicated; per-layer AllGather of transposed features.

v2 design (vs v1): the per-edge work is Q7-descriptor-bound (~6ns/desc), so
descriptors are minimized:
  - edge slots are laid out with partition = dst slot (dst's position within
    its 128-node block), so ed[dst] is a per-partition broadcast — the
    per-edge ed gather (1/3 of all descriptors in v1) is gone.
  - aggregation over edge slots = identity-matmul PSUM accumulation (sum
    over the free/slot dim), so the one-hot S tiles (+32MB/layer DMA) are
    gone.
  - buckets are exact-sized per block (graph known at compile time); pad
    slots (dst-degree imbalance) gather row 0 and are masked.
  - self-loops ride in-bucket as slot 0 of each partition.
  - sources are split into two overlapping table halves (int16 gather idx
    limit); overlap-band edges balance the two buckets per partition.

Per layer: esed (ed per own node, from resident hT) -> phase A (replicated:
full table h@W_ext -> DRAM, 260 cols) -> phase B per dst block: 2 gathers
(768B/edge), exm = mask*exp(lrelu(es+ed)), msg = [xh*exm | exm], psum +=
I @ msg[t] over slots; epilogue: head-mean/denominator, bias, ELU,
PE-transpose into next layer's hT. AllGather hT between layers.
"""
import dataclasses
import numpy as np

import concourse.bass as bass
import concourse.bacc as bacc
import concourse.mybir as mybir
import concourse.tile as tile

f32 = mybir.dt.float32
f32r = mybir.dt.bfloat16
i16 = mybir.dt.int16
ALU = mybir.AluOpType
ACTF = mybir.ActivationFunctionType

ROW = 384          # table row stride (elem_size for gather; 768B)
TCOL = 260         # used table cols: 256 xh + 2 es + 2 ed
HALF = 32768       # rows per gather window (int16 idx limit)


@dataclasses.dataclass(frozen=True)
class Cfg:
    n: int = 50000
    ncores: int = 8
    nlayers: int = 3
    hid: int = 128

    @property
    def nb(self):  return self.n // self.ncores
    @property
    def cpb(self):  return (self.nb + 127) // 128
    @property
    def npc(self):  return self.cpb * 128
    @property
    def npad(self): return self.ncores * self.npc
    @property
    def bbase(self): return self.npad - HALF   # start row of half-B window


# ---------------------------------------------------------------- host side

def pack_nodes(cfg, deg):
    """perm [N] -> slot. Per core: degree-desc sort, consecutive slots
    (so block b holds 128 nodes of similar degree across all cores)."""
    perm = np.full(cfg.n, -1, dtype=np.int64)
    for c in range(cfg.ncores):
        nodes = np.arange(c * cfg.nb, (c + 1) * cfg.nb)
        order = nodes[np.argsort(-deg[nodes], kind="stable")]
        perm[order] = c * cfg.npc + np.arange(cfg.nb)
    return perm


def preprocess(cfg, edge_index):
    """Build per-core gather idx + mask arrays and global per-block bucket
    sizes (T must be identical across cores: SPMD single program)."""
    src0 = np.asarray(edge_index[0], dtype=np.int64)
    dst0 = np.asarray(edge_index[1], dtype=np.int64)
    deg = np.bincount(dst0, minlength=cfg.n) + 1     # incl self-loop
    perm = pack_nodes(cfg, deg)

    ps = perm[src0]
    pd = perm[dst0]

    # per-slot edge lists: sort edges by dst slot
    order = np.argsort(pd, kind="stable")
    ps_s, pd_s = ps[order], pd[order]
    starts = np.searchsorted(pd_s, np.arange(cfg.npad + 1))

    inv = np.empty(cfg.npad, dtype=np.int64)   # slot -> node id (or -1)
    inv.fill(-1)
    inv[perm] = np.arange(cfg.n)

    # bucket rows per (core, block, partition, half)
    nA = np.zeros((cfg.ncores, cfg.cpb, 128), dtype=np.int32)
    nB = np.zeros((cfg.ncores, cfg.cpb, 128), dtype=np.int32)
    listsA = {}
    listsB = {}
    for c in range(cfg.ncores):
        for b in range(cfg.cpb):
            for p in range(128):
                slot = c * cfg.npc + b * 128 + p
                if inv[slot] < 0:
                    continue
                rows = [slot] + list(ps_s[starts[slot]:starts[slot + 1]])
                la, lb, flex = [], [], []
                for r in rows:
                    if r < cfg.bbase:
                        la.append(r)
                    elif r >= HALF:
                        lb.append(r)
                    else:
                        flex.append(r)
                # balance with flexible rows
                need_a = max(0, (len(rows) + 1) // 2 - len(la))
                take = min(need_a, len(flex))
                la += flex[:take]
                lb += flex[take:]
                listsA[(c, b, p)] = la
                listsB[(c, b, p)] = lb
                nA[c, b, p] = len(la)
                nB[c, b, p] = len(lb)

    # global per-block T (max across cores & partitions)
    TA = nA.max(axis=(0, 2)).astype(np.int64)   # [cpb]
    TB = nB.max(axis=(0, 2)).astype(np.int64)

    sumT = int((TA + TB).sum())
    # per-core idx arrays [T, 128] per (b,h) segment + mask [128, T, 2]
    idx_flat = np.zeros((cfg.ncores, sumT * 128), dtype=np.int16)
    mask = np.zeros((cfg.ncores, 128, sumT, 2), dtype=np.float32)
    seg_off = []   # per (b): (slot offset of segment start, TA, TB)
    off = 0
    for b in range(cfg.cpb):
        seg_off.append(off)
        off += int(TA[b] + TB[b])
    for c in range(cfg.ncores):
        for b in range(cfg.cpb):
            o = seg_off[b]
            ta, tb = int(TA[b]), int(TB[b])
            for p in range(128):
                la = listsA.get((c, b, p), [])
                lb = listsB.get((c, b, p), [])
                for t, r in enumerate(la):
                    idx_flat[c, (o + t) * 128 + p] = r
                    mask[c, p, o + t, :] = 1.0
                for t, r in enumerate(lb):
                    idx_flat[c, (o + ta + t) * 128 + p] = r - cfg.bbase
                    mask[c, p, o + ta + t, :] = 1.0
    return dict(perm=perm, TA=TA, TB=TB, seg_off=seg_off, sumT=sumT,
                idx_flat=idx_flat, mask=mask)


def wrap_rep(idx):
    """[K] int16 -> dma_gather wrapped layout [128, K/16]."""
    K = idx.shape[-1]
    w = idx.reshape(K // 16, 16).T.copy()       # [16, K/16]
    return np.tile(w, (8, 1)).copy()


def host_arrays(cfg, x, edge_index, params):
    import ml_dtypes
    bfl = ml_dtypes.bfloat16
    pp = preprocess(cfg, edge_index)
    perm = pp["perm"]

    xpad = np.zeros((cfg.npad, 128), dtype=np.float32)
    xpad[perm] = np.asarray(x, np.float32)
    xT_stack = np.ascontiguousarray(
        xpad.reshape(cfg.ncores, cfg.npc, 128).transpose(0, 2, 1)
        .reshape(cfg.ncores * 128, cfg.npc))

    w_ext = np.zeros((cfg.nlayers, 128, TCOL), dtype=np.float32)
    bias = np.zeros((cfg.nlayers, 128, 128), dtype=np.float32)
    for li, (W, a_s, a_d, b) in enumerate(params):
        W = np.asarray(W, np.float32)
        w_ext[li, :, :256] = W
        w_ext[li, :, 256] = W[:, :128] @ np.asarray(a_s, np.float32)[0]
        w_ext[li, :, 257] = W[:, 128:] @ np.asarray(a_s, np.float32)[1]
        w_ext[li, :, 258] = W[:, :128] @ np.asarray(a_d, np.float32)[0]
        w_ext[li, :, 259] = W[:, 128:] @ np.asarray(a_d, np.float32)[1]
        bias[li] = np.tile(np.asarray(b, np.float32)[None, :], (128, 1))

    # wrapped idx: concat per-(b) segments (each segment len 128*(TA+TB))
    per_core = []
    for c in range(cfg.ncores):
        idxw = wrap_rep(pp["idx_flat"][c])      # [128, sumT*8]
        per_core.append(dict(
            xT_stack=xT_stack.astype(bfl),
            xT_local=np.ascontiguousarray(
                xT_stack[c * 128:(c + 1) * 128]).astype(bfl),
            w_ext=w_ext.astype(bfl), bias=bias,
            ident=np.eye(128, dtype=np.float32),
            identb=np.eye(128, dtype=np.float32).astype(bfl),
            idxw=idxw,
            maskw=np.ascontiguousarray(
                pp["mask"][c].reshape(128, pp["sumT"] * 2)).astype(bfl),
        ))
    return pp, per_core


# -------------------------------------------------------------- device side

def build_nc(cfg, pp):
    nc = bacc.Bacc("TRN2", num_devices=cfg.ncores)
    NPC, CPB, NL, NSH = cfg.npc, cfg.cpb, cfg.nlayers, cfg.ncores
    TA, TB, seg_off, sumT = pp["TA"], pp["TB"], pp["seg_off"], pp["sumT"]
    TTCAP = int((TA + TB).max())
    NROWS = NSH * NPC

    xT_stack = nc.dram_tensor("xT_stack", [NSH * 128, NPC], f32r, kind="ExternalInput")
    xT_local = nc.dram_tensor("xT_local", [128, NPC], f32r, kind="ExternalInput")
    w_ext_in = nc.dram_tensor("w_ext", [NL, 128, TCOL], f32r, kind="ExternalInput")
    bias_in = nc.dram_tensor("bias", [NL, 128, 128], f32, kind="ExternalInput")
    ident_in = nc.dram_tensor("ident", [128, 128], f32, kind="ExternalInput")
    identb_in = nc.dram_tensor("identb", [128, 128], f32r, kind="ExternalInput")
    idx_in = nc.dram_tensor("idxw", [128, sumT * 8], i16, kind="ExternalInput")
    mask_in = nc.dram_tensor("maskw", [128, sumT * 2], f32r, kind="ExternalInput")
    out = nc.dram_tensor("out", [NPC, 128], f32, kind="ExternalOutput")

    with tile.TileContext(nc) as tc:
        with (
            tc.tile_pool(name="const", bufs=1) as constp,
            tc.tile_pool(name="dram", bufs=2, space="DRAM") as dramp,
            tc.tile_pool(name="hT", bufs=1) as hTp,
            tc.tile_pool(name="esed", bufs=1) as esedp,
            tc.tile_pool(name="slabA", bufs=2) as slabAp,
            tc.tile_pool(name="rowA", bufs=6) as rowAp,
            tc.tile_pool(name="g1", bufs=2) as g1p,
            tc.tile_pool(name="att", bufs=6) as attp,
            tc.tile_pool(name="ep", bufs=8) as epp,
            tc.tile_pool(name="psumE", bufs=1, space="PSUM") as psumEp,
            tc.tile_pool(name="psumA", bufs=3, space="PSUM") as psumAp,
            tc.tile_pool(name="psumB", bufs=3, space="PSUM") as psumBp,
            tc.tile_pool(name="psumT", bufs=1, space="PSUM") as psumTp,
        ):
            idx_sb = constp.tile([128, sumT * 8], i16)
            nc.sync.dma_start(idx_sb[:], idx_in.ap())
            mask_sb = constp.tile([128, sumT, 2], f32r)
            nc.sync.dma_start(mask_sb[:], mask_in.ap())
            w_sb = constp.tile([128, NL * TCOL], f32r)
            bias_sb = constp.tile([128, NL * 128], f32)
            for li in range(NL):
                nc.sync.dma_start(w_sb[:, li * TCOL:(li + 1) * TCOL], w_ext_in.ap()[li])
                nc.sync.dma_start(bias_sb[:, li * 128:(li + 1) * 128], bias_in.ap()[li])
            ident_sb = constp.tile([128, 128], f32)
            nc.sync.dma_start(ident_sb[:], ident_in.ap())
            identb_sb = constp.tile([128, 128], f32r)
            nc.sync.dma_start(identb_sb[:], identb_in.ap())

            # gather count registers (one per distinct 128*T)
            regs = {}
            for b in range(CPB):
                for T in (int(TA[b]), int(TB[b])):
                    if T and T not in regs:
                        regs[T] = nc.gpsimd.to_reg(128 * T)

            # resident own-transposed-h: double buffered across layers
            hT_buf = [hTp.tile([128, NPC], f32r, name=f"hT{i}") for i in range(2)]
            nc.sync.dma_start(hT_buf[0][:], xT_local.ap())
            esed_sb = esedp.tile([128, CPB, 2], f32r)

            for li in range(NL):
                w_l = w_sb[:, li * TCOL:(li + 1) * TCOL]
                bias_l = bias_sb[:, li * 128:(li + 1) * 128]
                last = li == NL - 1
                hin = hT_buf[li % 2]
                hout = hT_buf[(li + 1) % 2]

                # ---- esed: ed for own nodes (per dst partition)
                for j in range(CPB):
                    psE = psumEp.tile([128, 2], f32)
                    nc.tensor.matmul(
                        psE[:], hin[:, j * 128:(j + 1) * 128],
                        w_l[:, 258:260], start=True, stop=True)
                    nc.vector.tensor_copy(esed_sb[:, j, :], psE[:])

                # ---- phase A: full table (replicated on every core)
                table = dramp.tile([NROWS, ROW], f32r, tag="tab",
                                   name=f"table_l{li}")
                for s in range(NSH):
                    hTs = slabAp.tile([128, NPC], f32r)
                    if li == 0:
                        nc.sync.dma_start(
                            hTs[:], xT_stack.ap()[s * 128:(s + 1) * 128])
                    else:
                        nc.sync.dma_start(
                            hTs[:], hT_ag[s * 128:(s + 1) * 128])
                    for j in range(CPB):
                        psA = psumAp.tile([128, TCOL], f32)
                        nc.tensor.matmul(
                            psA[:], hTs[:, j * 128:(j + 1) * 128],
                            w_l[:, :TCOL], start=True, stop=True)
                        tA = rowAp.tile([128, TCOL], f32r)
                        nc.vector.tensor_copy(tA[:], psA[:])
                        base = s * NPC + j * 128
                        nc.sync.dma_start(
                            table[base:base + 128, 0:TCOL], tA[:])

                # ---- phase B: per dst block
                tabA = table[0:HALF]
                tabB = table[cfg.bbase:cfg.bbase + HALF]
                for b in range(CPB):
                    ta, tb = int(TA[b]), int(TB[b])
                    tt = ta + tb
                    o = seg_off[b]
                    g1 = g1p.tile([128, TTCAP, ROW], f32r, name="g1")
                    nc.gpsimd.dma_gather(
                        out_ap=g1[:, 0:ta, :], in_ap=tabA,
                        idxs_ap=idx_sb[:, o * 8:(o + ta) * 8],
                        num_idxs=128 * ta, num_idxs_reg=regs[ta],
                        elem_size=ROW, single_packet=False)
                    nc.gpsimd.dma_gather(
                        out_ap=g1[:, ta:tt, :], in_ap=tabB,
                        idxs_ap=idx_sb[:, (o + ta) * 8:(o + tt) * 8],
                        num_idxs=128 * tb, num_idxs_reg=regs[tb],
                        elem_size=ROW, single_packet=False)
                    # attention: exm = mask * exp(lrelu(es_src + ed_dst))
                    tat = attp.tile([128, TTCAP, 2], f32, tag="tat")
                    nc.vector.tensor_tensor(
                        out=tat[:, 0:tt, :], in0=g1[:, 0:tt, 256:258],
                        in1=esed_sb[:, b:b + 1, :].broadcast_to((128, tt, 2)),
                        op=ALU.add)
                    lk = attp.tile([128, TTCAP, 2], f32, tag="lk")
                    nc.vector.tensor_scalar(
                        out=lk[:, 0:tt, :], in0=tat[:, 0:tt, :],
                        scalar1=0.2, scalar2=None, op0=ALU.mult)
                    nc.vector.tensor_tensor(
                        out=lk[:, 0:tt, :], in0=lk[:, 0:tt, :],
                        in1=tat[:, 0:tt, :], op=ALU.max)
                    exm = attp.tile([128, TTCAP, 2], f32r, tag="exm")
                    nc.scalar.activation(exm[:, 0:tt, :], lk[:, 0:tt, :],
                                         ACTF.Exp)
                    nc.vector.tensor_tensor(
                        out=exm[:, 0:tt, :], in0=exm[:, 0:tt, :],
                        in1=mask_sb[:, o:o + tt, :], op=ALU.mult)
                    # msg in-place in g1: cols 0:256 *= exm, cols 256:258 = exm
                    for hh in range(2):
                        nc.vector.tensor_tensor(
                            out=g1[:, 0:tt, hh * 128:(hh + 1) * 128],
                            in0=g1[:, 0:tt, hh * 128:(hh + 1) * 128],
                            in1=exm[:, 0:tt, hh:hh + 1].broadcast_to(
                                (128, tt, 128)),
                            op=ALU.mult)
                    nc.vector.tensor_copy(g1[:, 0:tt, 256:258],
                                          exm[:, 0:tt, :])
                    psum = psumBp.tile([128, 258], f32)
                    for t in range(tt):
                        nc.tensor.matmul(
                            psum[:], identb_sb[:], g1[:, t, 0:258],
                            start=(t == 0), stop=(t == tt - 1))
                    # epilogue
                    rec = epp.tile([128, 2], f32, tag="rec")
                    nc.vector.tensor_scalar(
                        out=rec[:], in0=psum[:, 256:258], scalar1=1e-20,
                        scalar2=None, op0=ALU.add)
                    nc.vector.reciprocal(rec[:], rec[:])
                    h_blk = epp.tile([128, 128], f32, tag="hblk")
                    nc.vector.tensor_scalar(
                        out=h_blk[:], in0=psum[:, 0:128],
                        scalar1=rec[:, 0:1], scalar2=0.5,
                        op0=ALU.mult, op1=ALU.mult)
                    m1 = epp.tile([128, 128], f32, tag="m1")
                    nc.vector.tensor_scalar(
                        out=m1[:], in0=psum[:, 128:256],
                        scalar1=rec[:, 1:2], scalar2=0.5,
                        op0=ALU.mult, op1=ALU.mult)
                    nc.vector.tensor_tensor(
                        out=h_blk[:], in0=h_blk[:], in1=m1[:], op=ALU.add)
                    nc.vector.tensor_tensor(
                        out=h_blk[:], in0=h_blk[:], in1=bias_l, op=ALU.add)
                    if not last:
                        # ELU = (max(x,0)-1) + exp(min(x,0))
                        mn = epp.tile([128, 128], f32, tag="mn")
                        nc.vector.tensor_scalar(
                            out=mn[:], in0=h_blk[:], scalar1=0.0,
                            scalar2=None, op0=ALU.min)
                        emn = epp.tile([128, 128], f32, tag="emn")
                        nc.scalar.activation(emn[:], mn[:], ACTF.Exp)
                        nc.vector.tensor_scalar(
                            out=h_blk[:], in0=h_blk[:], scalar1=0.0,
                            scalar2=-1.0, op0=ALU.max, op1=ALU.add)
                        nc.vector.tensor_tensor(
                            out=h_blk[:], in0=h_blk[:], in1=emn[:],
                            op=ALU.add)
                        psT = psumTp.tile([128, 128], f32)
                        nc.tensor.transpose(psT[:], h_blk[:], ident_sb[:])
                        nc.vector.tensor_copy(
                            hout[:, b * 128:(b + 1) * 128], psT[:])
                    else:
                        nc.sync.dma_start(
                            out[b * 128:(b + 1) * 128, :], h_blk[:])
                if not last:
                    hT_loc = dramp.tile([128, NPC], f32r, tag="hloc")
                    nc.sync.dma_start(hT_loc[:], hout[:])
                    hT_ag = dramp.tile([NSH * 128, NPC], f32r, tag="hag",
                                       addr_space="Shared")
                    nc.gpsimd.collective_compute(
                        "AllGather", ALU.bypass,
                        replica_groups=[list(range(cfg.ncores))],
                        ins=[hT_loc.opt()], outs=[hT_ag.opt()])
    nc.compile()
    return nc


# ------------------------------------------------------------------ driver

def in_map(pc):
    return dict(xT_stack=pc["xT_stack"], xT_local=pc["xT_local"],
                w_ext=pc["w_ext"], bias=pc["bias"], ident=pc["ident"],
                identb=pc["identb"], idxw=pc["idxw"], maskw=pc["maskw"])


def run(cfg, x, edge_index, params, trace=False):
    from concourse.bass_utils import run_bass_kernel_spmd
    pp, per_core = host_arrays(cfg, x, edge_index, params)
    nc = build_nc(cfg, pp)
    in_maps = [in_map(pc) for pc in per_core]
    res = run_bass_kernel_spmd(
        nc, in_maps, core_ids=list(range(cfg.ncores)), trace=trace)
    full = np.concatenate([res.results[c]["out"] for c in range(cfg.ncores)])
    return full[pp["perm"]], res


# ------------------------------------------------------------- entry point

_CFG = Cfg()


def kernel(x, edge_index, W0, a_src0, a_dst0, b0, W1, a_src1, a_dst1, b1,
           W2, a_src2, a_dst2, b2):
    """Full-input GAT kernel: shards across 8 NeuronCores internally."""
    params = [(W0, a_src0, a_dst0, b0), (W1, a_src1, a_dst1, b1),
              (W2, a_src2, a_dst2, b2)]
    out, _ = run(_CFG, x, edge_index, params, trace=False)
    return np.asarray(out, dtype=np.float32)


# revision 17
# speedup vs baseline: 1.8756x; 1.4617x over previous
"""3-layer GAT (2 heads x 128) on 8 TRN2 NeuronCores — Bass/Tile kernel, v2.

Sharding: nodes partitioned across cores by destination (graph parallel);
weights replicated; per-layer AllGather of transposed features.

v2 design (vs v1): the per-edge work is Q7-descriptor-bound (~6ns/desc), so
descriptors are minimized:
  - edge slots are laid out with partition = dst slot (dst's position within
    its 128-node block), so ed[dst] is a per-partition broadcast — the
    per-edge ed gather (1/3 of all descriptors in v1) is gone.
  - aggregation over edge slots = identity-matmul PSUM accumulation (sum
    over the free/slot dim), so the one-hot S tiles (+32MB/layer DMA) are
    gone.
  - buckets are exact-sized per block (graph known at compile time); pad
    slots (dst-degree imbalance) gather row 0 and are masked.
  - self-loops ride in-bucket as slot 0 of each partition.
  - sources are split into two overlapping table halves (int16 gather idx
    limit); overlap-band edges balance the two buckets per partition.

Per layer: esed (ed per own node, from resident hT) -> phase A (replicated:
full table h@W_ext -> DRAM, 260 cols) -> phase B per dst block: 2 gathers
(768B/edge), exm = mask*exp(lrelu(es+ed)), msg = [xh*exm | exm], psum +=
I @ msg[t] over slots; epilogue: head-mean/denominator, bias, ELU,
PE-transpose into next layer's hT. AllGather hT between layers.
"""
import dataclasses
import numpy as np

import concourse.bass as bass
import concourse.bacc as bacc
import concourse.mybir as mybir
import concourse.tile as tile

f32 = mybir.dt.float32
f32r = mybir.dt.bfloat16
i16 = mybir.dt.int16
ALU = mybir.AluOpType
ACTF = mybir.ActivationFunctionType

ROW = 384          # table row stride (elem_size for gather; 768B)
TCOL = 260         # used table cols: 256 xh + 2 es + 2 ed
HALF = 32768       # rows per gather window (int16 idx limit)
NWIN = 3           # overlapping source windows (balance buckets)


@dataclasses.dataclass(frozen=True)
class Cfg:
    n: int = 50000
    ncores: int = 8
    nlayers: int = 3
    hid: int = 128

    @property
    def nb(self):  return self.n // self.ncores
    @property
    def cpb(self):  return (self.nb + 127) // 128
    @property
    def npc(self):  return self.cpb * 128
    @property
    def npad(self): return self.ncores * self.npc
    @property
    def wbase(self):
        # window start rows, evenly spread; last ends at npad
        return [round(w * (self.npad - HALF) / (NWIN - 1)) for w in range(NWIN)]


# ---------------------------------------------------------------- host side

def pack_nodes(cfg, deg):
    """perm [N] -> slot. Global degree-desc sort dealt round-robin to cores,
    so every core's block b holds nodes of near-identical degree (the
    per-(block,window) bucket size T is a cross-core max)."""
    order = np.argsort(-deg, kind="stable")
    perm = np.full(cfg.n, -1, dtype=np.int64)
    i = np.arange(cfg.n)
    perm[order] = (i % cfg.ncores) * cfg.npc + i // cfg.ncores
    return perm


def preprocess(cfg, edge_index):
    """Build per-core gather idx + mask arrays and global per-block bucket
    sizes (T must be identical across cores: SPMD single program)."""
    src0 = np.asarray(edge_index[0], dtype=np.int64)
    dst0 = np.asarray(edge_index[1], dtype=np.int64)
    deg = np.bincount(dst0, minlength=cfg.n) + 1     # incl self-loop
    perm = pack_nodes(cfg, deg)
    wbase = cfg.wbase

    ps = perm[src0]
    pd = perm[dst0]

    # per-slot edge lists: sort edges by dst slot
    order = np.argsort(pd, kind="stable")
    ps_s, pd_s = ps[order], pd[order]
    starts = np.searchsorted(pd_s, np.arange(cfg.npad + 1))

    inv = np.empty(cfg.npad, dtype=np.int64)   # slot -> node id (or -1)
    inv.fill(-1)
    inv[perm] = np.arange(cfg.n)

    # bucket rows per (core, block, partition, window); greedy balance of
    # flexible rows (windows overlap) to minimize per-window maxima
    nW = np.zeros((NWIN, cfg.ncores, cfg.cpb, 128), dtype=np.int32)
    lists = {}
    for c in range(cfg.ncores):
        for b in range(cfg.cpb):
            for p in range(128):
                slot = c * cfg.npc + b * 128 + p
                if inv[slot] < 0:
                    continue
                rows = [slot] + list(ps_s[starts[slot]:starts[slot + 1]])
                lw = [[] for _ in range(NWIN)]
                flex = []
                for r in rows:
                    elig = [w for w in range(NWIN)
                            if wbase[w] <= r < wbase[w] + HALF]
                    if len(elig) == 1:
                        lw[elig[0]].append(r)
                    else:
                        flex.append((r, elig))
                for r, elig in flex:
                    w = min(elig, key=lambda w: len(lw[w]))
                    lw[w].append(r)
                for w in range(NWIN):
                    lists[(w, c, b, p)] = lw[w]
                    nW[w, c, b, p] = len(lw[w])

    # global per-(block, window) T (max across cores & partitions)
    TW = nW.max(axis=(1, 3)).astype(np.int64)    # [NWIN, cpb]

    sumT = int(TW.sum())
    idx_flat = np.zeros((cfg.ncores, sumT * 128), dtype=np.int16)
    mask = np.zeros((cfg.ncores, 128, sumT, 2), dtype=np.float32)
    seg_off = []   # per (b): slot offset of block segment start
    off = 0
    for b in range(cfg.cpb):
        seg_off.append(off)
        off += int(TW[:, b].sum())
    for c in range(cfg.ncores):
        for b in range(cfg.cpb):
            o = seg_off[b]
            for w in range(NWIN):
                tw = int(TW[w, b])
                for p in range(128):
                    for t, r in enumerate(lists.get((w, c, b, p), [])):
                        idx_flat[c, (o + t) * 128 + p] = r - wbase[w]
                        mask[c, p, o + t, :] = 1.0
                o += tw
    return dict(perm=perm, TW=TW, seg_off=seg_off, sumT=sumT,
                idx_flat=idx_flat, mask=mask)


def wrap_rep(idx):
    """[K] int16 -> dma_gather wrapped layout [128, K/16]."""
    K = idx.shape[-1]
    w = idx.reshape(K // 16, 16).T.copy()       # [16, K/16]
    return np.tile(w, (8, 1)).copy()


def host_arrays(cfg, x, edge_index, params):
    import ml_dtypes
    bfl = ml_dtypes.bfloat16
    pp = preprocess(cfg, edge_index)
    perm = pp["perm"]

    xpad = np.zeros((cfg.npad, 128), dtype=np.float32)
    xpad[perm] = np.asarray(x, np.float32)
    xT_stack = np.ascontiguousarray(
        xpad.reshape(cfg.ncores, cfg.npc, 128).transpose(0, 2, 1)
        .reshape(cfg.ncores * 128, cfg.npc))

    w_ext = np.zeros((cfg.nlayers, 128, TCOL), dtype=np.float32)
    bias = np.zeros((cfg.nlayers, 128, 128), dtype=np.float32)
    for li, (W, a_s, a_d, b) in enumerate(params):
        W = np.asarray(W, np.float32)
        w_ext[li, :, :256] = W
        w_ext[li, :, 256] = W[:, :128] @ np.asarray(a_s, np.float32)[0]
        w_ext[li, :, 257] = W[:, 128:] @ np.asarray(a_s, np.float32)[1]
        w_ext[li, :, 258] = W[:, :128] @ np.asarray(a_d, np.float32)[0]
        w_ext[li, :, 259] = W[:, 128:] @ np.asarray(a_d, np.float32)[1]
        bias[li] = np.tile(np.asarray(b, np.float32)[None, :], (128, 1))

    # wrapped idx: concat per-(b) segments (each segment len 128*(TA+TB))
    per_core = []
    for c in range(cfg.ncores):
        idxw = wrap_rep(pp["idx_flat"][c])      # [128, sumT*8]
        per_core.append(dict(
            xT_stack=xT_stack.astype(bfl),
            xT_local=np.ascontiguousarray(
                xT_stack[c * 128:(c + 1) * 128]).astype(bfl),
            w_ext=w_ext.astype(bfl), bias=bias,
            ident=np.eye(128, dtype=np.float32),
            identb=np.eye(128, dtype=np.float32).astype(bfl),
            idxw=idxw,
            maskw=np.ascontiguousarray(
                pp["mask"][c].reshape(128, pp["sumT"] * 2)).astype(bfl),
        ))
    return pp, per_core


# -------------------------------------------------------------- device side

def build_nc(cfg, pp):
    nc = bacc.Bacc("TRN2", num_devices=cfg.ncores, num_swdge_queues=4)
    NPC, CPB, NL, NSH = cfg.npc, cfg.cpb, cfg.nlayers, cfg.ncores
    TW, seg_off, sumT = pp["TW"], pp["seg_off"], pp["sumT"]
    TTCAP = int(TW.sum(axis=0).max())
    NROWS = NSH * NPC
    SPLIT = (CPB // 2) * 128

    xT_stack = nc.dram_tensor("xT_stack", [NSH * 128, NPC], f32r, kind="ExternalInput")
    xT_local = nc.dram_tensor("xT_local", [128, NPC], f32r, kind="ExternalInput")
    w_ext_in = nc.dram_tensor("w_ext", [NL, 128, TCOL], f32r, kind="ExternalInput")
    bias_in = nc.dram_tensor("bias", [NL, 128, 128], f32, kind="ExternalInput")
    ident_in = nc.dram_tensor("ident", [128, 128], f32, kind="ExternalInput")
    identb_in = nc.dram_tensor("identb", [128, 128], f32r, kind="ExternalInput")
    idx_in = nc.dram_tensor("idxw", [128, sumT * 8], i16, kind="ExternalInput")
    mask_in = nc.dram_tensor("maskw", [128, sumT * 2], f32r, kind="ExternalInput")
    out = nc.dram_tensor("out", [NPC, 128], f32, kind="ExternalOutput")

    with tile.TileContext(nc) as tc:
        with (
            tc.tile_pool(name="const", bufs=1) as constp,
            tc.tile_pool(name="dram", bufs=2, space="DRAM") as dramp,
            tc.tile_pool(name="hT", bufs=1) as hTp,
            tc.tile_pool(name="esed", bufs=1) as esedp,
            tc.tile_pool(name="slabA", bufs=2) as slabAp,
            tc.tile_pool(name="rowA", bufs=6) as rowAp,
            tc.tile_pool(name="g1", bufs=3) as g1p,
            tc.tile_pool(name="att", bufs=6) as attp,
            tc.tile_pool(name="ep", bufs=8) as epp,
            tc.tile_pool(name="psumE", bufs=1, space="PSUM") as psumEp,
            tc.tile_pool(name="psumA", bufs=3, space="PSUM") as psumAp,
            tc.tile_pool(name="psumB", bufs=3, space="PSUM") as psumBp,
            tc.tile_pool(name="psumT", bufs=1, space="PSUM") as psumTp,
        ):
            idx_sb = constp.tile([128, sumT * 8], i16)
            nc.sync.dma_start(idx_sb[:], idx_in.ap())
            mask_sb = constp.tile([128, sumT, 2], f32r)
            nc.sync.dma_start(mask_sb[:], mask_in.ap())
            w_sb = constp.tile([128, NL * TCOL], f32r)
            bias_sb = constp.tile([128, NL * 128], f32)
            for li in range(NL):
                nc.sync.dma_start(w_sb[:, li * TCOL:(li + 1) * TCOL], w_ext_in.ap()[li])
                nc.sync.dma_start(bias_sb[:, li * 128:(li + 1) * 128], bias_in.ap()[li])
            ident_sb = constp.tile([128, 128], f32)
            nc.sync.dma_start(ident_sb[:], ident_in.ap())
            identb_sb = constp.tile([128, 128], f32r)
            nc.sync.dma_start(identb_sb[:], identb_in.ap())

            # gather count registers (one per distinct 128*T)
            regs = {}
            for b in range(CPB):
                for w in range(NWIN):
                    T = int(TW[w, b])
                    if T and T not in regs:
                        regs[T] = nc.gpsimd.to_reg(128 * T)

            # resident own-transposed-h: double buffered across layers
            hT_buf = [hTp.tile([128, NPC], f32r, name=f"hT{i}") for i in range(2)]
            nc.sync.dma_start(hT_buf[0][:], xT_local.ap())
            esed_sb = esedp.tile([128, CPB, 2], f32r)

            for li in range(NL):
                w_l = w_sb[:, li * TCOL:(li + 1) * TCOL]
                bias_l = bias_sb[:, li * 128:(li + 1) * 128]
                last = li == NL - 1
                hin = hT_buf[li % 2]
                hout = hT_buf[(li + 1) % 2]

                # ---- esed: ed for own nodes (per dst partition)
                for j in range(CPB):
                    psE = psumEp.tile([128, 2], f32)
                    nc.tensor.matmul(
                        psE[:], hin[:, j * 128:(j + 1) * 128],
                        w_l[:, 258:260], start=True, stop=True)
                    nc.vector.tensor_copy(esed_sb[:, j, :], psE[:])

                # ---- phase A: full table (replicated on every core)
                table = dramp.tile([NROWS, ROW], f32r, tag="tab",
                                   name=f"table_l{li}")
                for s in range(NSH):
                    hTs = slabAp.tile([128, NPC], f32r)
                    if li == 0:
                        nc.sync.dma_start(
                            hTs[:], xT_stack.ap()[s * 128:(s + 1) * 128])
                    else:
                        nc.sync.dma_start(
                            hTs[:, 0:SPLIT],
                            hT_ag1[s * 128:(s + 1) * 128])
                        nc.sync.dma_start(
                            hTs[:, SPLIT:NPC],
                            hT_ag2[s * 128:(s + 1) * 128])
                    for j in range(CPB):
                        psA = psumAp.tile([128, TCOL], f32)
                        nc.tensor.matmul(
                            psA[:], hTs[:, j * 128:(j + 1) * 128],
                            w_l[:, :TCOL], start=True, stop=True)
                        tA = rowAp.tile([128, TCOL], f32r)
                        if j % 2:
                            nc.scalar.activation(tA[:], psA[:], ACTF.Copy)
                        else:
                            nc.vector.tensor_copy(tA[:], psA[:])
                        base = s * NPC + j * 128
                        nc.sync.dma_start(
                            table[base:base + 128, 0:TCOL], tA[:])

                # ---- phase B: per dst block
                tabW = [table[wb:wb + HALF] for wb in cfg.wbase]
                qn = 0
                for b in range(CPB):
                    tws = [int(TW[w, b]) for w in range(NWIN)]
                    tt = sum(tws)
                    o = seg_off[b]
                    g1 = g1p.tile([128, TTCAP, ROW], f32r, name="g1")
                    so = 0
                    for w in range(NWIN):
                        tw = tws[w]
                        if tw == 0:
                            continue
                        nc.gpsimd.dma_gather(
                            out_ap=g1[:, so:so + tw, :], in_ap=tabW[w],
                            idxs_ap=idx_sb[:, (o + so) * 8:(o + so + tw) * 8],
                            num_idxs=128 * tw, num_idxs_reg=regs[tw],
                            elem_size=ROW, single_packet=False,
                            queue_num=qn)
                        qn = (qn + 1) % 4
                        so += tw
                    # attention: exm = mask * exp(lrelu(es_src + ed_dst))
                    tat = attp.tile([128, TTCAP, 2], f32, tag="tat")
                    nc.vector.tensor_tensor(
                        out=tat[:, 0:tt, :], in0=g1[:, 0:tt, 256:258],
                        in1=esed_sb[:, b:b + 1, :].broadcast_to((128, tt, 2)),
                        op=ALU.add)
                    lk = attp.tile([128, TTCAP, 2], f32, tag="lk")
                    nc.vector.tensor_scalar(
                        out=lk[:, 0:tt, :], in0=tat[:, 0:tt, :],
                        scalar1=0.2, scalar2=None, op0=ALU.mult)
                    nc.vector.tensor_tensor(
                        out=lk[:, 0:tt, :], in0=lk[:, 0:tt, :],
                        in1=tat[:, 0:tt, :], op=ALU.max)
                    exm = attp.tile([128, TTCAP, 2], f32r, tag="exm")
                    nc.scalar.activation(exm[:, 0:tt, :], lk[:, 0:tt, :],
                                         ACTF.Exp)
                    nc.vector.tensor_tensor(
                        out=exm[:, 0:tt, :], in0=exm[:, 0:tt, :],
                        in1=mask_sb[:, o:o + tt, :], op=ALU.mult)
                    # msg in-place in g1: cols 0:256 *= exm, cols 256:258 = exm
                    for hh in range(2):
                        nc.vector.tensor_tensor(
                            out=g1[:, 0:tt, hh * 128:(hh + 1) * 128],
                            in0=g1[:, 0:tt, hh * 128:(hh + 1) * 128],
                            in1=exm[:, 0:tt, hh:hh + 1].broadcast_to(
                                (128, tt, 128)),
                            op=ALU.mult)
                    nc.vector.tensor_copy(g1[:, 0:tt, 256:258],
                                          exm[:, 0:tt, :])
                    psum = psumBp.tile([128, 258], f32)
                    for t in range(tt):
                        nc.tensor.matmul(
                            psum[:], identb_sb[:], g1[:, t, 0:258],
                            start=(t == 0), stop=(t == tt - 1))
                    # epilogue
                    rec = epp.tile([128, 2], f32, tag="rec")
                    nc.vector.tensor_scalar(
                        out=rec[:], in0=psum[:, 256:258], scalar1=1e-20,
                        scalar2=None, op0=ALU.add)
                    nc.vector.reciprocal(rec[:], rec[:])
                    h_blk = epp.tile([128, 128], f32, tag="hblk")
                    nc.vector.tensor_scalar(
                        out=h_blk[:], in0=psum[:, 0:128],
                        scalar1=rec[:, 0:1], scalar2=0.5,
                        op0=ALU.mult, op1=ALU.mult)
                    m1 = epp.tile([128, 128], f32, tag="m1")
                    nc.vector.tensor_scalar(
                        out=m1[:], in0=psum[:, 128:256],
                        scalar1=rec[:, 1:2], scalar2=0.5,
                        op0=ALU.mult, op1=ALU.mult)
                    nc.vector.tensor_tensor(
                        out=h_blk[:], in0=h_blk[:], in1=m1[:], op=ALU.add)
                    nc.vector.tensor_tensor(
                        out=h_blk[:], in0=h_blk[:], in1=bias_l, op=ALU.add)
                    if not last:
                        # ELU = (max(x,0)-1) + exp(min(x,0))
                        mn = epp.tile([128, 128], f32, tag="mn")
                        nc.vector.tensor_scalar(
                            out=mn[:], in0=h_blk[:], scalar1=0.0,
                            scalar2=None, op0=ALU.min)
                        emn = epp.tile([128, 128], f32, tag="emn")
                        nc.scalar.activation(emn[:], mn[:], ACTF.Exp)
                        nc.vector.tensor_scalar(
                            out=h_blk[:], in0=h_blk[:], scalar1=0.0,
                            scalar2=-1.0, op0=ALU.max, op1=ALU.add)
                        nc.vector.tensor_tensor(
                            out=h_blk[:], in0=h_blk[:], in1=emn[:],
                            op=ALU.add)
                        psT = psumTp.tile([128, 128], f32)
                        nc.tensor.transpose(psT[:], h_blk[:], ident_sb[:])
                        nc.vector.tensor_copy(
                            hout[:, b * 128:(b + 1) * 128], psT[:])
                        if b == SPLIT // 128 - 1:
                            # first half of hout done: overlap its AllGather
                            # with the remaining blocks
                            hT_loc1 = dramp.tile([128, SPLIT], f32r,
                                                 tag="hloc1")
                            nc.sync.dma_start(hT_loc1[:], hout[:, 0:SPLIT])
                            hT_ag1 = dramp.tile([NSH * 128, SPLIT], f32r,
                                                tag="hag1",
                                                addr_space="Shared")
                            nc.gpsimd.collective_compute(
                                "AllGather", ALU.bypass,
                                replica_groups=[list(range(cfg.ncores))],
                                ins=[hT_loc1.opt()], outs=[hT_ag1.opt()])
                    else:
                        nc.sync.dma_start(
                            out[b * 128:(b + 1) * 128, :], h_blk[:])
                if not last:
                    hT_loc2 = dramp.tile([128, NPC - SPLIT], f32r,
                                         tag="hloc2")
                    nc.sync.dma_start(hT_loc2[:], hout[:, SPLIT:NPC])
                    hT_ag2 = dramp.tile([NSH * 128, NPC - SPLIT], f32r,
                                        tag="hag2", addr_space="Shared")
                    nc.gpsimd.collective_compute(
                        "AllGather", ALU.bypass,
                        replica_groups=[list(range(cfg.ncores))],
                        ins=[hT_loc2.opt()], outs=[hT_ag2.opt()])
    nc.compile()
    return nc


# ------------------------------------------------------------------ driver

def in_map(pc):
    return dict(xT_stack=pc["xT_stack"], xT_local=pc["xT_local"],
                w_ext=pc["w_ext"], bias=pc["bias"], ident=pc["ident"],
                identb=pc["identb"], idxw=pc["idxw"], maskw=pc["maskw"])


def run(cfg, x, edge_index, params, trace=False):
    from concourse.bass_utils import run_bass_kernel_spmd
    pp, per_core = host_arrays(cfg, x, edge_index, params)
    nc = build_nc(cfg, pp)
    in_maps = [in_map(pc) for pc in per_core]
    res = run_bass_kernel_spmd(
        nc, in_maps, core_ids=list(range(cfg.ncores)), trace=trace)
    full = np.concatenate([res.results[c]["out"] for c in range(cfg.ncores)])
    return full[pp["perm"]], res


# ------------------------------------------------------------- entry point

_CFG = Cfg()


def kernel(x, edge_index, W0, a_src0, a_dst0, b0, W1, a_src1, a_dst1, b1,
           W2, a_src2, a_dst2, b2):
    """Full-input GAT kernel: shards across 8 NeuronCores internally."""
    params = [(W0, a_src0, a_dst0, b0), (W1, a_src1, a_dst1, b1),
              (W2, a_src2, a_dst2, b2)]
    out, _ = run(_CFG, x, edge_index, params, trace=False)
    return np.asarray(out, dtype=np.float32)


# revision 23
# speedup vs baseline: 2.1112x; 1.1256x over previous
"""3-layer GAT (2 heads x 128) on 8 TRN2 NeuronCores — Bass/Tile kernel, v2.

Sharding: nodes partitioned across cores by destination (graph parallel);
weights replicated; per-layer AllGather of transposed features.

v2 design (vs v1): the per-edge work is Q7-descriptor-bound (~6ns/desc), so
descriptors are minimized:
  - edge slots are laid out with partition = dst slot (dst's position within
    its 128-node block), so ed[dst] is a per-partition broadcast — the
    per-edge ed gather (1/3 of all descriptors in v1) is gone.
  - aggregation over edge slots = identity-matmul PSUM accumulation (sum
    over the free/slot dim), so the one-hot S tiles (+32MB/layer DMA) are
    gone.
  - buckets are exact-sized per block (graph known at compile time); pad
    slots (dst-degree imbalance) gather row 0 and are masked.
  - self-loops ride in-bucket as slot 0 of each partition.
  - sources are split into two overlapping table halves (int16 gather idx
    limit); overlap-band edges balance the two buckets per partition.

Per layer: esed (ed per own node, from resident hT) -> phase A (replicated:
full table h@W_ext -> DRAM, 260 cols) -> phase B per dst block: 2 gathers
(768B/edge), exm = mask*exp(lrelu(es+ed)), msg = [xh*exm | exm], psum +=
I @ msg[t] over slots; epilogue: head-mean/denominator, bias, ELU,
PE-transpose into next layer's hT. AllGather hT between layers.
"""
import dataclasses
import numpy as np

import concourse.bass as bass
import concourse.bacc as bacc
import concourse.mybir as mybir
import concourse.tile as tile

f32 = mybir.dt.float32
f32r = mybir.dt.bfloat16
i16 = mybir.dt.int16
ALU = mybir.AluOpType
ACTF = mybir.ActivationFunctionType

ROW = 384          # table row stride (elem_size for gather; 768B)
TCOL = 260         # used table cols: 256 xh + 2 es + 2 ed
HALF = 32768       # rows per gather window (int16 idx limit)
NWIN = 3           # overlapping source windows (balance buckets)
DUP = 17408        # rows [0, DUP) are duplicated after the main table so
                   # window 2 can reach them (near-full 2-window coverage)


@dataclasses.dataclass(frozen=True)
class Cfg:
    n: int = 50000
    ncores: int = 8
    nlayers: int = 3
    hid: int = 128

    @property
    def nb(self):  return self.n // self.ncores
    @property
    def cpb(self):  return (self.nb + 127) // 128
    @property
    def npc(self):  return self.cpb * 128
    @property
    def npad(self): return self.ncores * self.npc
    @property
    def wbase(self):
        # window start rows over [main | dup of rows 0:DUP]
        return [0, DUP, 2 * DUP]


# ---------------------------------------------------------------- host side

def pack_nodes(cfg, deg):
    """perm [N] -> slot. Global degree-desc sort dealt round-robin to cores,
    so every core's block b holds nodes of near-identical degree (the
    per-(block,window) bucket size T is a cross-core max)."""
    order = np.argsort(-deg, kind="stable")
    perm = np.full(cfg.n, -1, dtype=np.int64)
    i = np.arange(cfg.n)
    perm[order] = (i % cfg.ncores) * cfg.npc + i // cfg.ncores
    return perm


def preprocess(cfg, edge_index):
    """Build per-core gather idx + mask arrays and global per-block bucket
    sizes (T must be identical across cores: SPMD single program)."""
    src0 = np.asarray(edge_index[0], dtype=np.int64)
    dst0 = np.asarray(edge_index[1], dtype=np.int64)
    deg = np.bincount(dst0, minlength=cfg.n) + 1     # incl self-loop
    perm = pack_nodes(cfg, deg)
    wbase = cfg.wbase

    ps = perm[src0]
    pd = perm[dst0]

    # per-slot edge lists: sort edges by dst slot
    order = np.argsort(pd, kind="stable")
    ps_s, pd_s = ps[order], pd[order]
    starts = np.searchsorted(pd_s, np.arange(cfg.npad + 1))

    inv = np.empty(cfg.npad, dtype=np.int64)   # slot -> node id (or -1)
    inv.fill(-1)
    inv[perm] = np.arange(cfg.n)

    # bucket rows per (core, block, partition, window); greedy balance of
    # flexible rows (windows overlap) to minimize per-window maxima
    nW = np.zeros((NWIN, cfg.ncores, cfg.cpb, 128), dtype=np.int32)
    lists = {}
    for c in range(cfg.ncores):
        for b in range(cfg.cpb):
            for p in range(128):
                slot = c * cfg.npc + b * 128 + p
                if inv[slot] < 0:
                    continue
                rows = [slot] + list(ps_s[starts[slot]:starts[slot + 1]])
                lw = [[] for _ in range(NWIN)]
                flex = []
                for r in rows:
                    # positions of row r: r (main) and npad+r (dup, r < DUP)
                    elig = [w for w in range(NWIN)
                            if wbase[w] <= r < wbase[w] + HALF
                            or (r < DUP
                                and wbase[w] <= cfg.npad + r < wbase[w] + HALF)]
                    if len(elig) == 1:
                        lw[elig[0]].append(r)
                    else:
                        flex.append((r, elig))
                for r, elig in flex:
                    w = min(elig, key=lambda w: len(lw[w]))
                    lw[w].append(r)
                for w in range(NWIN):
                    lists[(w, c, b, p)] = lw[w]
                    nW[w, c, b, p] = len(lw[w])

    # global per-(block, window) T (max across cores & partitions)
    TW = nW.max(axis=(1, 3)).astype(np.int64)    # [NWIN, cpb]

    sumT = int(TW.sum())
    idx_flat = np.zeros((cfg.ncores, sumT * 128), dtype=np.int16)
    mask = np.zeros((cfg.ncores, 128, sumT, 2), dtype=np.float32)
    seg_off = []   # per (b): slot offset of block segment start
    off = 0
    for b in range(cfg.cpb):
        seg_off.append(off)
        off += int(TW[:, b].sum())
    for c in range(cfg.ncores):
        for b in range(cfg.cpb):
            o = seg_off[b]
            for w in range(NWIN):
                tw = int(TW[w, b])
                for p in range(128):
                    for t, r in enumerate(lists.get((w, c, b, p), [])):
                        pos = r
                        if not (wbase[w] <= pos < wbase[w] + HALF):
                            pos = cfg.npad + r    # dup copy
                        idx_flat[c, (o + t) * 128 + p] = pos - wbase[w]
                        mask[c, p, o + t, :] = 1.0
                o += tw
    return dict(perm=perm, TW=TW, seg_off=seg_off, sumT=sumT,
                idx_flat=idx_flat, mask=mask)


def wrap_rep(idx):
    """[K] int16 -> dma_gather wrapped layout [128, K/16]."""
    K = idx.shape[-1]
    w = idx.reshape(K // 16, 16).T.copy()       # [16, K/16]
    return np.tile(w, (8, 1)).copy()


def host_arrays(cfg, x, edge_index, params):
    import ml_dtypes
    bfl = ml_dtypes.bfloat16
    pp = preprocess(cfg, edge_index)
    perm = pp["perm"]

    xpad = np.zeros((cfg.npad, 128), dtype=np.float32)
    xpad[perm] = np.asarray(x, np.float32)
    xT_stack = np.ascontiguousarray(
        xpad.reshape(cfg.ncores, cfg.npc, 128).transpose(0, 2, 1)
        .reshape(cfg.ncores * 128, cfg.npc))

    w_ext = np.zeros((cfg.nlayers, 128, TCOL), dtype=np.float32)
    bias = np.zeros((cfg.nlayers, 128, 128), dtype=np.float32)
    for li, (W, a_s, a_d, b) in enumerate(params):
        W = np.asarray(W, np.float32)
        w_ext[li, :, :256] = W
        w_ext[li, :, 256] = W[:, :128] @ np.asarray(a_s, np.float32)[0]
        w_ext[li, :, 257] = W[:, 128:] @ np.asarray(a_s, np.float32)[1]
        w_ext[li, :, 258] = W[:, :128] @ np.asarray(a_d, np.float32)[0]
        w_ext[li, :, 259] = W[:, 128:] @ np.asarray(a_d, np.float32)[1]
        bias[li] = np.tile(np.asarray(b, np.float32)[None, :], (128, 1))

    # wrapped idx: concat per-(b) segments (each segment len 128*(TA+TB))
    per_core = []
    for c in range(cfg.ncores):
        idxw = wrap_rep(pp["idx_flat"][c])      # [128, sumT*8]
        per_core.append(dict(
            xT_stack=xT_stack.astype(bfl),
            xT_local=np.ascontiguousarray(
                xT_stack[c * 128:(c + 1) * 128]).astype(bfl),
            w_ext=w_ext.astype(bfl), bias=bias,
            ident=np.eye(128, dtype=np.float32),
            identb=np.eye(128, dtype=np.float32).astype(bfl),
            idxw=idxw,
            maskw=np.ascontiguousarray(
                pp["mask"][c].reshape(128, pp["sumT"] * 2)).astype(bfl),
        ))
    return pp, per_core


# -------------------------------------------------------------- device side

def build_nc(cfg, pp):
    nc = bacc.Bacc("TRN2", num_devices=cfg.ncores, num_swdge_queues=4)
    NPC, CPB, NL, NSH = cfg.npc, cfg.cpb, cfg.nlayers, cfg.ncores
    TW, seg_off, sumT = pp["TW"], pp["seg_off"], pp["sumT"]
    TTCAP = int(TW.sum(axis=0).max())
    NROWS = NSH * NPC + DUP       # main table + duplicated low rows
    SPLIT = (CPB // 2) * 128

    xT_stack = nc.dram_tensor("xT_stack", [NSH * 128, NPC], f32r, kind="ExternalInput")
    xT_local = nc.dram_tensor("xT_local", [128, NPC], f32r, kind="ExternalInput")
    w_ext_in = nc.dram_tensor("w_ext", [NL, 128, TCOL], f32r, kind="ExternalInput")
    bias_in = nc.dram_tensor("bias", [NL, 128, 128], f32, kind="ExternalInput")
    ident_in = nc.dram_tensor("ident", [128, 128], f32, kind="ExternalInput")
    identb_in = nc.dram_tensor("identb", [128, 128], f32r, kind="ExternalInput")
    idx_in = nc.dram_tensor("idxw", [128, sumT * 8], i16, kind="ExternalInput")
    mask_in = nc.dram_tensor("maskw", [128, sumT * 2], f32r, kind="ExternalInput")
    out = nc.dram_tensor("out", [NPC, 128], f32, kind="ExternalOutput")

    with tile.TileContext(nc) as tc:
        with (
            tc.tile_pool(name="const", bufs=1) as constp,
            tc.tile_pool(name="dram", bufs=2, space="DRAM") as dramp,
            tc.tile_pool(name="hT", bufs=1) as hTp,
            tc.tile_pool(name="esed", bufs=1) as esedp,
            tc.tile_pool(name="slabA", bufs=2) as slabAp,
            tc.tile_pool(name="rowA", bufs=6) as rowAp,
            tc.tile_pool(name="g1", bufs=3) as g1p,
            tc.tile_pool(name="att", bufs=6) as attp,
            tc.tile_pool(name="ep", bufs=8) as epp,
            tc.tile_pool(name="psumE", bufs=1, space="PSUM") as psumEp,
            tc.tile_pool(name="psumA", bufs=3, space="PSUM") as psumAp,
            tc.tile_pool(name="psumB", bufs=3, space="PSUM") as psumBp,
            tc.tile_pool(name="psumT", bufs=1, space="PSUM") as psumTp,
        ):
            idx_sb = constp.tile([128, sumT * 8], i16)
            nc.sync.dma_start(idx_sb[:], idx_in.ap())
            mask_sb = constp.tile([128, sumT, 2], f32r)
            nc.sync.dma_start(mask_sb[:], mask_in.ap())
            w_sb = constp.tile([128, NL * TCOL], f32r)
            bias_sb = constp.tile([128, NL * 128], f32)
            for li in range(NL):
                nc.sync.dma_start(w_sb[:, li * TCOL:(li + 1) * TCOL], w_ext_in.ap()[li])
                nc.sync.dma_start(bias_sb[:, li * 128:(li + 1) * 128], bias_in.ap()[li])
            ident_sb = constp.tile([128, 128], f32)
            nc.sync.dma_start(ident_sb[:], ident_in.ap())
            identb_sb = constp.tile([128, 128], f32r)
            nc.sync.dma_start(identb_sb[:], identb_in.ap())

            # gather count registers (one per distinct 128*T)
            regs = {}
            for b in range(CPB):
                for w in range(NWIN):
                    T = int(TW[w, b])
                    if T and T not in regs:
                        regs[T] = nc.gpsimd.to_reg(128 * T)

            # resident own-transposed-h: double buffered across layers
            hT_buf = [hTp.tile([128, NPC], f32r, name=f"hT{i}") for i in range(2)]
            nc.sync.dma_start(hT_buf[0][:], xT_local.ap())
            esed_sb = esedp.tile([128, CPB, 2], f32r)

            for li in range(NL):
                w_l = w_sb[:, li * TCOL:(li + 1) * TCOL]
                bias_l = bias_sb[:, li * 128:(li + 1) * 128]
                last = li == NL - 1
                hin = hT_buf[li % 2]
                hout = hT_buf[(li + 1) % 2]

                # ---- esed: ed for own nodes (per dst partition)
                for j in range(CPB):
                    psE = psumEp.tile([128, 2], f32)
                    nc.tensor.matmul(
                        psE[:], hin[:, j * 128:(j + 1) * 128],
                        w_l[:, 258:260], start=True, stop=True)
                    nc.vector.tensor_copy(esed_sb[:, j, :], psE[:])

                # ---- phase A: full table (replicated on every core).
                # For li>0 it runs in two passes: part 1 consumes hT_ag1
                # (available mid-B of the previous layer, so the scheduler
                # can overlap it), part 2 consumes hT_ag2.
                table = dramp.tile([NROWS, ROW], f32r, tag="tab",
                                   name=f"table_l{li}")
                JSPLIT = SPLIT // 128

                def a_block(hTs, col0, s, j):
                    psA = psumAp.tile([128, TCOL], f32)
                    nc.tensor.matmul(
                        psA[:], hTs[:, j * 128 - col0:(j + 1) * 128 - col0],
                        w_l[:, :TCOL], start=True, stop=True)
                    tA = rowAp.tile([128, TCOL], f32r)
                    if j % 2:
                        nc.scalar.activation(tA[:], psA[:], ACTF.Copy)
                    else:
                        nc.vector.tensor_copy(tA[:], psA[:])
                    base = s * NPC + j * 128
                    nc.sync.dma_start(table[base:base + 128, 0:TCOL], tA[:])
                    if base < DUP:
                        nc.sync.dma_start(
                            table[NSH * NPC + base:NSH * NPC + base + 128,
                                  0:TCOL], tA[:])

                if li == 0:
                    for s in range(NSH):
                        hTs = slabAp.tile([128, NPC], f32r, tag="s0")
                        nc.sync.dma_start(
                            hTs[:], xT_stack.ap()[s * 128:(s + 1) * 128])
                        for j in range(CPB):
                            a_block(hTs, 0, s, j)
                else:
                    for s in range(NSH):
                        hTs = slabAp.tile([128, SPLIT], f32r, tag="s1")
                        nc.sync.dma_start(
                            hTs[:], hT_ag1[s * 128:(s + 1) * 128])
                        for j in range(JSPLIT):
                            a_block(hTs, 0, s, j)
                    for s in range(NSH):
                        hTs = slabAp.tile([128, NPC - SPLIT], f32r, tag="s2")
                        nc.sync.dma_start(
                            hTs[:], hT_ag2[s * 128:(s + 1) * 128])
                        for j in range(JSPLIT, CPB):
                            a_block(hTs, SPLIT, s, j)

                # ---- phase B: per dst block
                tabW = [table[wb:wb + HALF] for wb in cfg.wbase]
                qn = 0
                for b in range(CPB):
                    tws = [int(TW[w, b]) for w in range(NWIN)]
                    tt = sum(tws)
                    o = seg_off[b]
                    g1 = g1p.tile([128, TTCAP, ROW], f32r, name="g1")
                    so = 0
                    for w in range(NWIN):
                        tw = tws[w]
                        if tw == 0:
                            continue
                        nc.gpsimd.dma_gather(
                            out_ap=g1[:, so:so + tw, :], in_ap=tabW[w],
                            idxs_ap=idx_sb[:, (o + so) * 8:(o + so + tw) * 8],
                            num_idxs=128 * tw, num_idxs_reg=regs[tw],
                            elem_size=ROW, single_packet=False,
                            queue_num=qn)
                        qn = (qn + 1) % 4
                        so += tw
                    # attention: exm = mask * exp(lrelu(es_src + ed_dst))
                    tat = attp.tile([128, TTCAP, 2], f32, tag="tat")
                    nc.vector.tensor_tensor(
                        out=tat[:, 0:tt, :], in0=g1[:, 0:tt, 256:258],
                        in1=esed_sb[:, b:b + 1, :].broadcast_to((128, tt, 2)),
                        op=ALU.add)
                    lk = attp.tile([128, TTCAP, 2], f32, tag="lk")
                    nc.vector.tensor_scalar(
                        out=lk[:, 0:tt, :], in0=tat[:, 0:tt, :],
                        scalar1=0.2, scalar2=None, op0=ALU.mult)
                    nc.vector.tensor_tensor(
                        out=lk[:, 0:tt, :], in0=lk[:, 0:tt, :],
                        in1=tat[:, 0:tt, :], op=ALU.max)
                    exm = attp.tile([128, TTCAP, 2], f32r, tag="exm")
                    nc.scalar.activation(exm[:, 0:tt, :], lk[:, 0:tt, :],
                                         ACTF.Exp)
                    nc.vector.tensor_tensor(
                        out=exm[:, 0:tt, :], in0=exm[:, 0:tt, :],
                        in1=mask_sb[:, o:o + tt, :], op=ALU.mult)
                    # msg in-place in g1: cols 0:256 *= exm, cols 256:258 = exm
                    for hh in range(2):
                        nc.vector.tensor_tensor(
                            out=g1[:, 0:tt, hh * 128:(hh + 1) * 128],
                            in0=g1[:, 0:tt, hh * 128:(hh + 1) * 128],
                            in1=exm[:, 0:tt, hh:hh + 1].broadcast_to(
                                (128, tt, 128)),
                            op=ALU.mult)
                    nc.vector.tensor_copy(g1[:, 0:tt, 256:258],
                                          exm[:, 0:tt, :])
                    psum = psumBp.tile([128, 258], f32)
                    for t in range(tt):
                        nc.tensor.matmul(
                            psum[:], identb_sb[:], g1[:, t, 0:258],
                            start=(t == 0), stop=(t == tt - 1))
                    # epilogue
                    rec = epp.tile([128, 2], f32, tag="rec")
                    nc.vector.tensor_scalar(
                        out=rec[:], in0=psum[:, 256:258], scalar1=1e-20,
                        scalar2=None, op0=ALU.add)
                    nc.vector.reciprocal(rec[:], rec[:])
                    h_blk = epp.tile([128, 128], f32, tag="hblk")
                    nc.vector.tensor_scalar(
                        out=h_blk[:], in0=psum[:, 0:128],
                        scalar1=rec[:, 0:1], scalar2=0.5,
                        op0=ALU.mult, op1=ALU.mult)
                    m1 = epp.tile([128, 128], f32, tag="m1")
                    nc.vector.tensor_scalar(
                        out=m1[:], in0=psum[:, 128:256],
                        scalar1=rec[:, 1:2], scalar2=0.5,
                        op0=ALU.mult, op1=ALU.mult)
                    nc.vector.tensor_tensor(
                        out=h_blk[:], in0=h_blk[:], in1=m1[:], op=ALU.add)
                    nc.vector.tensor_tensor(
                        out=h_blk[:], in0=h_blk[:], in1=bias_l, op=ALU.add)
                    if not last:
                        # ELU = (max(x,0)-1) + exp(min(x,0))
                        mn = epp.tile([128, 128], f32, tag="mn")
                        nc.vector.tensor_scalar(
                            out=mn[:], in0=h_blk[:], scalar1=0.0,
                            scalar2=None, op0=ALU.min)
                        emn = epp.tile([128, 128], f32, tag="emn")
                        nc.scalar.activation(emn[:], mn[:], ACTF.Exp)
                        nc.vector.tensor_scalar(
                            out=h_blk[:], in0=h_blk[:], scalar1=0.0,
                            scalar2=-1.0, op0=ALU.max, op1=ALU.add)
                        nc.vector.tensor_tensor(
                            out=h_blk[:], in0=h_blk[:], in1=emn[:],
                            op=ALU.add)
                        psT = psumTp.tile([128, 128], f32)
                        nc.tensor.transpose(psT[:], h_blk[:], ident_sb[:])
                        nc.vector.tensor_copy(
                            hout[:, b * 128:(b + 1) * 128], psT[:])
                        if b == SPLIT // 128 - 1:
                            # first half of hout done: overlap its AllGather
                            # with the remaining blocks
                            hT_loc1 = dramp.tile([128, SPLIT], f32r,
                                                 tag="hloc1")
                            nc.sync.dma_start(hT_loc1[:], hout[:, 0:SPLIT])
                            hT_ag1 = dramp.tile([NSH * 128, SPLIT], f32r,
                                                tag="hag1",
                                                addr_space="Shared")
                            nc.gpsimd.collective_compute(
                                "AllGather", ALU.bypass,
                                replica_groups=[list(range(cfg.ncores))],
                                ins=[hT_loc1.opt()], outs=[hT_ag1.opt()])
                    else:
                        nc.sync.dma_start(
                            out[b * 128:(b + 1) * 128, :], h_blk[:])
                if not last:
                    hT_loc2 = dramp.tile([128, NPC - SPLIT], f32r,
                                         tag="hloc2")
                    nc.sync.dma_start(hT_loc2[:], hout[:, SPLIT:NPC])
                    hT_ag2 = dramp.tile([NSH * 128, NPC - SPLIT], f32r,
                                        tag="hag2", addr_space="Shared")
                    nc.gpsimd.collective_compute(
                        "AllGather", ALU.bypass,
                        replica_groups=[list(range(cfg.ncores))],
                        ins=[hT_loc2.opt()], outs=[hT_ag2.opt()])
    nc.compile()
    return nc


# ------------------------------------------------------------------ driver

def in_map(pc):
    return dict(xT_stack=pc["xT_stack"], xT_local=pc["xT_local"],
                w_ext=pc["w_ext"], bias=pc["bias"], ident=pc["ident"],
                identb=pc["identb"], idxw=pc["idxw"], maskw=pc["maskw"])


def run(cfg, x, edge_index, params, trace=False):
    from concourse.bass_utils import run_bass_kernel_spmd
    pp, per_core = host_arrays(cfg, x, edge_index, params)
    nc = build_nc(cfg, pp)
    in_maps = [in_map(pc) for pc in per_core]
    res = run_bass_kernel_spmd(
        nc, in_maps, core_ids=list(range(cfg.ncores)), trace=trace)
    full = np.concatenate([res.results[c]["out"] for c in range(cfg.ncores)])
    return full[pp["perm"]], res


# ------------------------------------------------------------- entry point

_CFG = Cfg()


def kernel(x, edge_index, W0, a_src0, a_dst0, b0, W1, a_src1, a_dst1, b1,
           W2, a_src2, a_dst2, b2):
    """Full-input GAT kernel: shards across 8 NeuronCores internally."""
    params = [(W0, a_src0, a_dst0, b0), (W1, a_src1, a_dst1, b1),
              (W2, a_src2, a_dst2, b2)]
    out, _ = run(_CFG, x, edge_index, params, trace=False)
    return np.asarray(out, dtype=np.float32)
